# revision 2
# baseline (speedup 1.0000x reference)
"""CTRNN cell (6 Euler unfolds) on 8 Trainium2 NeuronCores.

Math (per unfold, 6x):
    f     = tanh([x, s] @ W + b)
    s_new = s + 0.1 * (-s + f)  = 0.9*s + 0.1*f

Strategy (v2 — chunk-pipelined bf16):
  - Data-parallel over batch: B=8192 -> 1024 rows/core, no cross-core
    communication. Host does the cheap numpy transposes/packing.
  - Everything transposed on-chip (feature dim on SBUF partitions, batch
    on the free dim) so state feeds the tensor engine as the moving
    operand and W slices are directly the stationary lhsT.
  - pre = x @ W_top computed once; per-unfold matmuls in *delta* form:
    one persistent PSUM accumulator per (m-tile, chunk) holds
    pre + s_k @ W_bot across all unfolds via psum += (f_k - s_k) @ (0.1*W_bot).
    7-logical-matmul FLOP floor, PSUM never restarts.
  - All matmul operands in bf16 (x, 10*s cast, W_top, 0.1*W_bot, tmp).
    PSUM accumulation stays fp32; the state v = 10*s is kept in f32 so
    the recurrence never loses precision. bf16 operand rounding is ~0.4%
    RMS per contribution -> ~1e-2 worst-element abs error vs the 6.3e-2
    budget (deterministic: same seed every run). Halves input DMA bytes
    and enables FWL fast weight loads.
  - Chunk pipelining: each 1024-batch round is split into two 512-batch
    chunk-rounds (c0/c1). While the PE runs chunk c1's 16 matmuls, the
    ACT engine tanh's chunk c0 and DVE produces chunk c0's tmp, so the
    tensor engine never waits at unfold boundaries.
  - v += tmp state update runs as SWDGE accumulate-DMA (SBUF->SBUF,
    accum_op=add) on the otherwise-idle DMA engines, keeping the DVE
    under the per-round tensor-engine budget.
  - Input DMA is need-ordered: per-(k-tile, chunk) pieces issued on the
    two HWDGE rings + SWDGE in exactly consumption order, so the first
    matmul can start ~1.7us in and data streams just ahead of the PE.
  - Final unfold emits 10*s_6 = 0.9*v + f in one fused DVE op per chunk
    and DMAs it out immediately; the host scales by 0.1 while unpacking
    (it already scales s by 10 while packing).
  - A junk-matmul + junk-tanh warm-up starts the HAM un-throttle clock
    and pre-loads the ACT tanh table while the first inputs stream in.
"""

import numpy as np

UNFOLDS = 6
DT = 0.1
B, D, N = 8192, 512, 512
NCORES = 8
BC = B // NCORES          # batch rows per core
CH = 512                  # chunk: matmul moving free dim (1 PSUM bank)
NCH = BC // CH            # 2
P = 128
KT = D // P               # k-tiles (4) for each of W_top / W_bot
MT = N // P               # m-tiles of the output dim (4)

_compiled_nc = None

# v += tmp update: "dma" = SWDGE accumulate-DMA, "split" = vector+gpsimd
VADD_MODE = "dma"


def _build_nc():
    import concourse.bass as bass  # noqa: F401
    import concourse.bacc as bacc
    import concourse.tile as tile
    from concourse import mybir

    f32 = mybir.dt.float32
    bf16 = mybir.dt.bfloat16
    MULT = mybir.AluOpType.mult
    ADD = mybir.AluOpType.add
    TANH = mybir.ActivationFunctionType.Tanh

    nc = bacc.Bacc("TRN2", target_bir_lowering=False, debug=False)

    xB = nc.dram_tensor("xB", [P, KT * BC], bf16, kind="ExternalInput").ap()
    sP = nc.dram_tensor("sP", [P, KT * BC], f32, kind="ExternalInput").ap()
    wtB = nc.dram_tensor("wtB", [P, KT * N], bf16, kind="ExternalInput").ap()
    wbB = nc.dram_tensor("wbB", [P, KT * N], bf16, kind="ExternalInput").ap()
    bias = nc.dram_tensor("bias", [N], f32, kind="ExternalInput").ap()
    outT = nc.dram_tensor("outT", [N, BC], f32, kind="ExternalOutput").ap()

    with tile.TileContext(nc) as tc:
        with (
            tc.tile_pool(name="weights", bufs=1) as wpool,
            tc.tile_pool(name="data", bufs=1) as data,
            tc.tile_pool(name="tmp", bufs=2) as tmpp,
            tc.tile_pool(name="fpool", bufs=2) as fpool,
            tc.tile_pool(name="psum", bufs=1, space="PSUM") as psump,
        ):
            # ---- warm-up: start HAM clock + load the ACT tanh table ------
            junk = wpool.tile([P, CH], bf16, tag="junk", name="junk")
            nc.gpsimd.memset(junk[:], 0)
            junk2 = wpool.tile([P, CH], bf16, tag="junk2", name="junk2")
            nc.scalar.activation(junk2[:], junk[:], TANH)

            # ---- input DMAs: need-ordered pieces on 3 rings --------------
            # sync ring: x (j,c) pieces in consumption order, then 2 late
            # s pieces; scalar ring: W_top then 0.1*W_bot k-tiles; SWDGE:
            # bias + the first 6 s pieces. Every piece lands just ahead of
            # the matmul that consumes it.
            x_sb = data.tile([P, KT * BC], bf16, tag="x", name="x_sb")
            v_sb = data.tile([P, KT * BC], f32, tag="v", name="v_sb")
            sr_sb = data.tile([P, KT * BC], bf16, tag="sr", name="sr_sb")
            wt_sb = wpool.tile([P, KT * N], bf16, tag="wt", name="wt_sb")
            wb_sb = wpool.tile([P, KT * N], bf16, tag="wb", name="wb_sb")
            bias_sb = wpool.tile([P, MT], f32, tag="bias", name="bias_sb")

            def xsl(j, c):
                return slice(j * BC + c * CH, j * BC + (c + 1) * CH)

            for c in range(NCH):
                for j in range(KT):
                    nc.sync.dma_start(x_sb[:, xsl(j, c)], xB[:, xsl(j, c)])
            for j in range(KT):
                nc.scalar.dma_start(wt_sb[:, j * N:(j + 1) * N],
                                    wtB[:, j * N:(j + 1) * N])
            for j in range(KT):
                nc.scalar.dma_start(wb_sb[:, j * N:(j + 1) * N],
                                    wbB[:, j * N:(j + 1) * N])
            nc.gpsimd.dma_start(bias_sb[:], bias.rearrange("(m p) -> p m", p=P))
            s_pieces = [(j, c) for c in range(NCH) for j in range(KT)]
            for j, c in s_pieces[:6]:
                nc.gpsimd.dma_start(v_sb[:, xsl(j, c)], sP[:, xsl(j, c)])
            for j, c in s_pieces[6:]:
                nc.sync.dma_start(v_sb[:, xsl(j, c)], sP[:, xsl(j, c)])

            # bf16 cast of 10*s for the init matmul (2x_2P single-src DVE
            # copies, run while the PE chews on the x rounds)
            for j, c in s_pieces:
                nc.vector.tensor_copy(sr_sb[:, xsl(j, c)], v_sb[:, xsl(j, c)])

            # ---- persistent PSUM accumulators ----------------------------
            ps = [psump.tile([P, BC], f32, tag=f"ps{m}", name=f"ps{m}")
                  for m in range(MT)]

            # junk matmuls keep the PE busy from t=0 so HAM un-throttles to
            # 2.4 GHz by the time real matmuls stream (overwritten by the
            # first start=True matmul per bank).
            for r in range(12):
                nc.tensor.matmul(
                    ps[r % MT][:, 0:CH],
                    lhsT=junk[:, 0:P], rhs=junk[:, 0:CH],
                    start=True, stop=True, skip_group_check=True,
                )

            def mm_round(w_sb, rhs_of, c, start, stop):
                for j in range(KT):
                    for m in range(MT):
                        nc.tensor.matmul(
                            ps[m][:, c * CH:(c + 1) * CH],
                            lhsT=w_sb[:, j * N + m * P: j * N + (m + 1) * P],
                            rhs=rhs_of(j, c),
                            start=(start and j == 0),
                            stop=(stop and j == KT - 1),
                            skip_group_check=True,
                        )

            # init: psum = x @ W_top + (10*s0) @ (0.1*W_bot), chunk-split
            for c in range(NCH):
                mm_round(wt_sb, lambda j, c: x_sb[:, xsl(j, c)], c,
                         start=True, stop=False)
            for c in range(NCH):
                mm_round(wb_sb, lambda j, c: sr_sb[:, xsl(j, c)], c,
                         start=False, stop=False)

            # ---- unfolds (chunk-pipelined) -------------------------------
            # state kept scaled: v = 10*s (f32). Per chunk-round:
            #   f = tanh(psum + bias)            (ACT, 512 wide)
            #   tmp = f - 0.1*v                  (DVE stt -> bf16)
            #   psum += tmp @ (0.1*W_bot)        (16 matmuls)
            #   v += tmp                         (accum-DMA or DVE/GpSimd)
            for k in range(UNFOLDS - 1):
                tmp_t = [tmpp.tile([P, BC], bf16, tag=f"tmp{j}",
                                   name=f"tmp{k}_{j}")
                         for j in range(MT)]
                f_t = [fpool.tile([P, BC], f32, tag=f"f{m}", name=f"f{k}_{m}")
                       for m in range(MT)]
                for c in range(NCH):
                    cs = slice(c * CH, (c + 1) * CH)
                    for m in range(MT):
                        nc.scalar.activation(
                            f_t[m][:, cs], ps[m][:, cs], TANH,
                            bias=bias_sb[:, m:m + 1], scale=1.0,
                        )
                    for m in range(MT):
                        nc.vector.scalar_tensor_tensor(
                            tmp_t[m][:, cs], v_sb[:, xsl(m, c)], -DT,
                            f_t[m][:, cs], op0=MULT, op1=ADD,
                        )
                    mm_round(wb_sb, lambda j, c: tmp_t[j][:, cs], c,
                             start=False, stop=(k == UNFOLDS - 2))
                    # state update, off the critical path
                    for m in range(MT):
                        if VADD_MODE == "dma":
                            nc.gpsimd.dma_start(
                                v_sb[:, xsl(m, c)], tmp_t[m][:, cs],
                                accum_op=ADD,
                            )
                        else:
                            eng = nc.vector if m == 0 else nc.gpsimd
                            eng.tensor_tensor(
                                v_sb[:, xsl(m, c)], v_sb[:, xsl(m, c)],
                                tmp_t[m][:, cs], ADD,
                            )

            # ---- final unfold + store ------------------------------------
            # out10 = 0.9*v + f = 10*s_6, written in place over v and
            # DMA'd per (m, chunk); host multiplies by 0.1 while unpacking.
            f_t = [fpool.tile([P, BC], f32, tag=f"f{m}", name=f"f5_{m}")
                   for m in range(MT)]
            out_eng = [nc.sync, nc.scalar, nc.gpsimd, nc.sync]
            for c in range(NCH):
                cs = slice(c * CH, (c + 1) * CH)
                for m in range(MT):
                    nc.scalar.activation(
                        f_t[m][:, cs], ps[m][:, cs], TANH,
                        bias=bias_sb[:, m:m + 1], scale=1.0,
                    )
                for m in range(MT):
                    nc.vector.scalar_tensor_tensor(
                        v_sb[:, xsl(m, c)], v_sb[:, xsl(m, c)], 0.9,
                        f_t[m][:, cs], op0=MULT, op1=ADD,
                    )
                    out_eng[m].dma_start(
                        outT[m * P:(m + 1) * P, c * CH:(c + 1) * CH],
                        v_sb[:, xsl(m, c)],
                    )

    nc.compile()
    return nc


def _get_nc():
    global _compiled_nc
    if _compiled_nc is None:
        _compiled_nc = _build_nc()
    return _compiled_nc


def _ktile_pack(a, free):
    """(KT*P, free) -> (P, KT*free) with k-tiles side by side."""
    return np.ascontiguousarray(
        a.reshape(KT, P, free).transpose(1, 0, 2).reshape(P, -1))


def make_in_maps(x, s, W, b):
    """Shard + pack host-side. x/W in bf16 (matmul operands), s in f32
    scaled by 10 (state); all packed as (128, KT*free) k-tile layouts so
    every DMA piece has contiguous per-partition runs."""
    import ml_dtypes
    bf16 = ml_dtypes.bfloat16

    xT = np.ascontiguousarray(x.T)            # (D, B)
    sT = np.ascontiguousarray(s.T)            # (N, B)
    Wt = _ktile_pack(W[:D].astype(bf16), N)
    Wb = _ktile_pack((DT * W[D:]).astype(bf16), N)
    in_maps = []
    for c in range(NCORES):
        sl = slice(c * BC, (c + 1) * BC)
        in_maps.append({
            "xB": _ktile_pack(xT[:, sl].astype(bf16), BC),
            "sP": _ktile_pack(10.0 * sT[:, sl], BC),
            "wtB": Wt,
            "wbB": Wb,
            "bias": b,
        })
    return in_maps


def kernel(**inputs):
    from concourse.bass_utils import run_bass_kernel_spmd

    x = np.asarray(inputs["inputs"], dtype=np.float32)
    s = np.asarray(inputs["state"], dtype=np.float32)
    W = np.ascontiguousarray(np.asarray(inputs["W"], dtype=np.float32))
    b = np.ascontiguousarray(np.asarray(inputs["bias"], dtype=np.float32))

    in_maps = make_in_maps(x, s, W, b)
    nc = _get_nc()
    res = run_bass_kernel_spmd(nc, in_maps, list(range(NCORES))).results
    outT = np.concatenate([res[c]["outT"] for c in range(NCORES)], axis=1)
    out = np.ascontiguousarray(DT * outT.T).astype(np.float32)
    return (out, out)


# revision 3
# speedup vs baseline: 1.0583x; 1.0583x over previous
"""CTRNN cell (6 Euler unfolds) on 8 Trainium2 NeuronCores.

Math (per unfold, 6x):
    f     = tanh([x, s] @ W + b)
    s_new = s + 0.1 * (-s + f)  = 0.9*s + 0.1*f

Strategy (v3 — chunk-pipelined, bf16 datapath, f32 PSUM):
  - Data-parallel over batch: B=8192 -> 1024 rows/core, no cross-core
    communication. Host does the cheap numpy transposes/packing.
  - Everything transposed on-chip (feature dim on SBUF partitions, batch
    on the free dim) so state feeds the tensor engine as the moving
    operand and W slices are directly the stationary lhsT.
  - pre = x @ W_top computed once; per-unfold matmuls in *delta* form:
    one persistent PSUM accumulator per (m-tile, chunk) holds
    pre + s_k @ W_bot across all unfolds via psum += (f_k - s_k) @ (0.1*W_bot).
    7-logical-matmul FLOP floor, PSUM never restarts.
  - bf16 everywhere except PSUM + the stored output: x, W_top, 0.1*W_bot,
    the state v = 10*s, f = tanh(psum), and tmp = f - 0.1*v. PSUM
    accumulation stays fp32. Errors are deterministic (fixed seed):
    measured ~1e-2 absmax vs the 6.3e-2 gate.
  - bf16 state makes every DVE op all-16-bit -> 2X_1PORT dual-pumped
    (2 elem/cycle), so one vector engine covers tmp AND the v += tmp
    state update (~6.2us/round) under the tensor-engine round time
    (~7.3us), with ACT at ~5.8us/round. No GpSimd / accum-DMA involved
    (both measured slower).
  - Chunk pipelining: each 1024-batch round is split into two 512-batch
    chunk-rounds (c0/c1). While the PE runs chunk c1's 16 matmuls, the
    ACT engine tanh's chunk c0 and DVE produces chunk c0's tmp, so the
    tensor engine never waits at unfold boundaries.
  - Input DMA is need-ordered: per-(k-tile, chunk) pieces issued on the
    two HWDGE rings + SWDGE in exactly consumption order, so the first
    matmul can start ~1.7us in and data streams just ahead of the PE.
    Total input is only 2.5 MB/core (bf16).
  - Final unfold emits 10*s_6 = 0.9*v + f (fused DVE op, f32 out) per
    (m, chunk) piece and DMAs it out immediately; the host scales by 0.1
    while unpacking (it already scales s by 10 while packing).
  - A junk-matmul + junk-tanh warm-up starts the HAM un-throttle clock
    and pre-loads the ACT tanh table while the first inputs stream in.
"""

import numpy as np

UNFOLDS = 6
DT = 0.1
B, D, N = 8192, 512, 512
NCORES = 8
BC = B // NCORES          # batch rows per core
CH = 512                  # chunk: matmul moving free dim (1 PSUM bank)
NCH = BC // CH            # 2
P = 128
KT = D // P               # k-tiles (4) for each of W_top / W_bot
MT = N // P               # m-tiles of the output dim (4)

_compiled_nc = None


def _build_nc():
    import concourse.bass as bass  # noqa: F401
    import concourse.bacc as bacc
    import concourse.tile as tile
    from concourse import mybir

    f32 = mybir.dt.float32
    bf16 = mybir.dt.bfloat16
    MULT = mybir.AluOpType.mult
    ADD = mybir.AluOpType.add
    TANH = mybir.ActivationFunctionType.Tanh

    nc = bacc.Bacc("TRN2", target_bir_lowering=False, debug=False)

    xB = nc.dram_tensor("xB", [P, KT * BC], bf16, kind="ExternalInput").ap()
    sB = nc.dram_tensor("sB", [P, KT * BC], bf16, kind="ExternalInput").ap()
    wtB = nc.dram_tensor("wtB", [P, KT * N], bf16, kind="ExternalInput").ap()
    wbB = nc.dram_tensor("wbB", [P, KT * N], bf16, kind="ExternalInput").ap()
    bias = nc.dram_tensor("bias", [N], f32, kind="ExternalInput").ap()
    outT = nc.dram_tensor("outT", [N, BC], f32, kind="ExternalOutput").ap()

    with tile.TileContext(nc) as tc:
        with (
            tc.tile_pool(name="weights", bufs=1) as wpool,
            tc.tile_pool(name="data", bufs=1) as data,
            tc.tile_pool(name="tmp", bufs=2) as tmpp,
            tc.tile_pool(name="fpool", bufs=2) as fpool,
            tc.tile_pool(name="opool", bufs=1) as opool,
            tc.tile_pool(name="psum", bufs=1, space="PSUM") as psump,
        ):
            # ---- warm-up: start HAM clock + load the ACT tanh table ------
            junk = wpool.tile([P, CH], bf16, tag="junk", name="junk")
            nc.gpsimd.memset(junk[:], 0)
            junk2 = wpool.tile([P, CH], bf16, tag="junk2", name="junk2")
            nc.scalar.activation(junk2[:], junk[:], TANH)

            # ---- input DMAs: need-ordered pieces on 3 rings --------------
            # sync ring: x (j,c) pieces in consumption order; scalar ring:
            # W_top then 0.1*W_bot k-tiles; SWDGE: bias + s pieces. Every
            # piece lands just ahead of the matmul that consumes it.
            x_sb = data.tile([P, KT * BC], bf16, tag="x", name="x_sb")
            v_sb = data.tile([P, KT * BC], bf16, tag="v", name="v_sb")
            wt_sb = wpool.tile([P, KT * N], bf16, tag="wt", name="wt_sb")
            wb_sb = wpool.tile([P, KT * N], bf16, tag="wb", name="wb_sb")
            bias_sb = wpool.tile([P, MT], f32, tag="bias", name="bias_sb")

            def xsl(j, c):
                return slice(j * BC + c * CH, j * BC + (c + 1) * CH)

            for c in range(NCH):
                for j in range(KT):
                    nc.sync.dma_start(x_sb[:, xsl(j, c)], xB[:, xsl(j, c)])
            for j in range(KT):
                nc.scalar.dma_start(wt_sb[:, j * N:(j + 1) * N],
                                    wtB[:, j * N:(j + 1) * N])
            for j in range(KT):
                nc.scalar.dma_start(wb_sb[:, j * N:(j + 1) * N],
                                    wbB[:, j * N:(j + 1) * N])
            nc.gpsimd.dma_start(bias_sb[:], bias.rearrange("(m p) -> p m", p=P))
            for c in range(NCH):
                for j in range(KT):
                    nc.gpsimd.dma_start(v_sb[:, xsl(j, c)], sB[:, xsl(j, c)])

            # ---- persistent PSUM accumulators ----------------------------
            ps = [psump.tile([P, BC], f32, tag=f"ps{m}", name=f"ps{m}")
                  for m in range(MT)]

            # junk matmuls keep the PE busy from t=0 so HAM un-throttles to
            # 2.4 GHz by the time real matmuls stream (overwritten by the
            # first start=True matmul per bank).
            for r in range(12):
                nc.tensor.matmul(
                    ps[r % MT][:, 0:CH],
                    lhsT=junk[:, 0:P], rhs=junk[:, 0:CH],
                    start=True, stop=True, skip_group_check=True,
                )

            def mm_round(w_sb, rhs_of, c, start, stop):
                for j in range(KT):
                    for m in range(MT):
                        nc.tensor.matmul(
                            ps[m][:, c * CH:(c + 1) * CH],
                            lhsT=w_sb[:, j * N + m * P: j * N + (m + 1) * P],
                            rhs=rhs_of(j, c),
                            start=(start and j == 0),
                            stop=(stop and j == KT - 1),
                            skip_group_check=True,
                        )

            # init: psum = x @ W_top + (10*s0) @ (0.1*W_bot), chunk-split
            for c in range(NCH):
                mm_round(wt_sb, lambda j, c: x_sb[:, xsl(j, c)], c,
                         start=True, stop=False)
            for c in range(NCH):
                mm_round(wb_sb, lambda j, c: v_sb[:, xsl(j, c)], c,
                         start=False, stop=False)

            # ---- unfolds (chunk-pipelined) -------------------------------
            # state kept scaled: v = 10*s (bf16). Per chunk-round:
            #   f = tanh(psum + bias)            (ACT, 512 wide, bf16 out)
            #   tmp = f - 0.1*v                  (DVE stt, 2X_1P)
            #   psum += tmp @ (0.1*W_bot)        (16 matmuls)
            #   v += tmp                         (DVE tt, 2X_1P)
            for k in range(UNFOLDS - 1):
                tmp_t = [tmpp.tile([P, BC], bf16, tag=f"tmp{j}",
                                   name=f"tmp{k}_{j}")
                         for j in range(MT)]
                f_t = [fpool.tile([P, BC], bf16, tag=f"f{m}", name=f"f{k}_{m}")
                       for m in range(MT)]
                for c in range(NCH):
                    cs = slice(c * CH, (c + 1) * CH)
                    for m in range(MT):
                        nc.scalar.activation(
                            f_t[m][:, cs], ps[m][:, cs], TANH,
                            bias=bias_sb[:, m:m + 1], scale=1.0,
                        )
                    for m in range(MT):
                        nc.vector.scalar_tensor_tensor(
                            tmp_t[m][:, cs], v_sb[:, xsl(m, c)], -DT,
                            f_t[m][:, cs], op0=MULT, op1=ADD,
                        )
                    mm_round(wb_sb, lambda j, c: tmp_t[j][:, cs], c,
                             start=False, stop=(k == UNFOLDS - 2))
                    # state update, off the critical path
                    for m in range(MT):
                        nc.vector.tensor_tensor(
                            v_sb[:, xsl(m, c)], v_sb[:, xsl(m, c)],
                            tmp_t[m][:, cs], ADD,
                        )

            # ---- final unfold + store ------------------------------------
            # out10 = 0.9*v + f = 10*s_6 (f32), DMA'd per (m, chunk);
            # host multiplies by 0.1 while unpacking.
            f_t = [fpool.tile([P, BC], bf16, tag=f"f{m}", name=f"f5_{m}")
                   for m in range(MT)]
            o_t = [opool.tile([P, BC], f32, tag=f"o{m}", name=f"o{m}")
                   for m in range(MT)]
            out_eng = [nc.sync, nc.scalar, nc.gpsimd, nc.sync]
            for c in range(NCH):
                cs = slice(c * CH, (c + 1) * CH)
                for m in range(MT):
                    nc.scalar.activation(
                        f_t[m][:, cs], ps[m][:, cs], TANH,
                        bias=bias_sb[:, m:m + 1], scale=1.0,
                    )
                for m in range(MT):
                    nc.vector.scalar_tensor_tensor(
                        o_t[m][:, cs], v_sb[:, xsl(m, c)], 0.9,
                        f_t[m][:, cs], op0=MULT, op1=ADD,
                    )
                    out_eng[m].dma_start(
                        outT[m * P:(m + 1) * P, c * CH:(c + 1) * CH],
                        o_t[m][:, cs],
                    )

    nc.compile()
    return nc


def _get_nc():
    global _compiled_nc
    if _compiled_nc is None:
        _compiled_nc = _build_nc()
    return _compiled_nc


def _ktile_pack(a, free):
    """(KT*P, free) -> (P, KT*free) with k-tiles side by side."""
    return np.ascontiguousarray(
        a.reshape(KT, P, free).transpose(1, 0, 2).reshape(P, -1))


def make_in_maps(x, s, W, b):
    """Shard + pack host-side. x/W/10*s in bf16 (matmul operands + state);
    all packed as (128, KT*free) k-tile layouts so every DMA piece has
    contiguous per-partition runs."""
    import ml_dtypes
    bf16 = ml_dtypes.bfloat16

    xT = np.ascontiguousarray(x.T)            # (D, B)
    sT = np.ascontiguousarray(s.T)            # (N, B)
    Wt = _ktile_pack(W[:D].astype(bf16), N)
    Wb = _ktile_pack((DT * W[D:]).astype(bf16), N)
    in_maps = []
    for c in range(NCORES):
        sl = slice(c * BC, (c + 1) * BC)
        in_maps.append({
            "xB": _ktile_pack(xT[:, sl].astype(bf16), BC),
            "sB": _ktile_pack((10.0 * sT[:, sl]).astype(bf16), BC),
            "wtB": Wt,
            "wbB": Wb,
            "bias": b,
        })
    return in_maps


def kernel(**inputs):
    from concourse.bass_utils import run_bass_kernel_spmd

    x = np.asarray(inputs["inputs"], dtype=np.float32)
    s = np.asarray(inputs["state"], dtype=np.float32)
    W = np.ascontiguousarray(np.asarray(inputs["W"], dtype=np.float32))
    b = np.ascontiguousarray(np.asarray(inputs["bias"], dtype=np.float32))

    in_maps = make_in_maps(x, s, W, b)
    nc = _get_nc()
    res = run_bass_kernel_spmd(nc, in_maps, list(range(NCORES))).results
    outT = np.concatenate([res[c]["outT"] for c in range(NCORES)], axis=1)
    out = np.ascontiguousarray(DT * outT.T).astype(np.float32)
    return (out, out)
